# revision 1
# baseline (speedup 1.0000x reference)
"""Trainium2 Bass kernel for nn_CrossEntropyLossWeight3.

Math: per row b of predict/target [B,16]:
  probs   = softmax(predict[b])
  pre     = argmax(predict[b]);  tar = argmax(target[b])
  w       = 0 if pre==tar else penalty[tar, pre]
  loss_b  = w * probs[pre]
out = mean_b(loss_b)

Key identities used on-device:
  probs[pre]   = exp(max(x)) / sum(exp(x))      (softmax at its own argmax)
  penalty[i,j] = max(c_i,c_j)/(c_i+c_j) with distinct per-class counts c;
  with u = c[pre], v = c[tar]:  w = (u != v) * max(u,v)/(u+v).
  counts/1000 (9 bits, exact) are embedded into the low mantissa bits of the
  raw inputs, so one fused embed+segmented-max DVE scan per tensor yields
  the row max together with its argmax's class count (<= 2^-14 relative
  perturbation). Two more fused custom DVE ops evaluate the whole per-row
  weight formula straight from the embedded maxima:
    WNUM = (u!=v) * max(u,v)        SPD = u + v
  so loss_b = WNUM * exp(m) / (SPD * sumexp).

v4 engine balance (per [128, 256*16] tile, DMA budget ~11.7us):
  - DVE     : two embed+segmax scans (~12us); per-block WNUM/SPD/recip
  - ACT     : exp(predict) (the only table -> no ACT table reloads), exp(m)
  - GPSIMD  : contiguous-halves pairwise-add tree for row sums + 3 big TTs
  - DMA     : predict on the SP (sync) HWDGE ring, target on the ACT ring
  - blocks  : tiles 0-5 share one stats block (formula overlaps tiles 6-7
              streaming); tiles 6-7 form a small tail block whose formula
              runs on DVE (idle by then) to shrink the tail
Sharding: pure data parallel over 8 cores (batch split); each core returns
per-partition partial sums [128,1]; host reduces and divides by B.
"""

import sys

sys.path.insert(0, "/opt/trn_rl_repo")

import numpy as np

import concourse.bass as bass
import concourse.bacc as bacc
import concourse.tile as tile
from concourse import mybir
from concourse.bass_utils import run_bass_kernel_spmd

B, W = 2097152, 16
NCORES = 8
BS = B // NCORES          # rows per core
P = 128                   # SBUF partitions
R = 256                   # rows per partition per tile
F = R * W                 # free elems per partition per tile
TILE_ROWS = P * R
NT = BS // TILE_ROWS      # tiles per core
BLK_A = 6                 # tiles in the big (overlapped) stats block
BLK_B = NT - BLK_A        # tiles in the small tail stats block

LABELS_NUM_COUNT = [500000, 120000, 80000, 45000, 30000, 250000, 15000, 9000,
                    60000, 7000, 180000, 22000, 11000, 95000, 5000, 40000]

f32 = mybir.dt.float32
bf16 = mybir.dt.bfloat16
u32 = mybir.dt.uint32
AX = mybir.AxisListType
OP = mybir.AluOpType
ACT = mybir.ActivationFunctionType

PAYLOAD_BITS = 9          # counts/1000 <= 500 fits in 9 bits exactly
PAYLOAD_MASK = (1 << PAYLOAD_BITS) - 1
F_2P23 = 8388608.0        # bit pattern 0x4B000000; OR'ing these bits onto the
                          # 9-bit payload makes the exact float 2^23 + payload
F_2P24 = 16777216.0


def _register_custom_ops():
    """Three runtime-registered custom DVE ops.

    EMBMAX_SEG_ANT: fused "embed payload + segmented max" scan (see v2/v3
      history): body = Scan(MAX, ((x|c)^c)|pay, _subdim_step=Zero) over a
      [P, S, 16] view; stride-0 out leaves per-segment maxima in [P, S].
      The OR/XOR form avoids an AND with 0xFFFFFE00 (NaN bit pattern).
    WNUM_ANT(me, mt; s0=mask, s1=2^23): with u' = (me & mask) | bits(s1),
      v' = (mt & mask) | bits(s1)  (both exact floats 2^23 + count):
      out = (u' != v') * (max(u',v') - 2^23) = (u!=v)*max(u,v).
    SPD_ANT(me, mt; s0, s1, imm2=2^24): out = u' + v' - 2^24 = u + v.
    """
    import numpy as np_

    from concourse.dve_spec import (
        Spec, Src0, Src1, C0, C1, C2, Bin, AluOp, lower, ne, maxx, Zero,
    )
    from concourse.dve_ops import (
        DveOp,
        OPS,
        CUSTOM_DVE_SPECS,
        _SUB_OPCODE_FOR_NAME,
        _CUSTOM_DVE_ROW_BASE,
        _COMPILE_CACHE,
    )
    from concourse.dve_uop import DveOpSpec
    import concourse.dve_spec as ds

    def reg(name, spec, rd1):
        for o in OPS:
            if o.name == name:
                return o
        shas = {}
        for ver in ("v3", "v4"):
            uops = lower(spec, ver=ver)
            s = DveOpSpec(
                name=name,
                opcode=_CUSTOM_DVE_ROW_BASE + len(OPS),
                uops=uops,
                rd1_en=rd1,
            )
            shas[ver] = s.sha(ver)
        op = DveOp(name, spec, subdim=False, uops_sha=shas)
        _SUB_OPCODE_FOR_NAME[name] = _CUSTOM_DVE_ROW_BASE + len(OPS)
        OPS.append(op)
        CUSTOM_DVE_SPECS[name] = spec
        return op

    embed_expr = Bin(
        AluOp.BITWISE_OR,
        Bin(AluOp.BITWISE_XOR, Bin(AluOp.BITWISE_OR, Src0, C0), C0),
        Src1,
    )

    def _ref_embmax(in0, in1, s0, s1, imm2):
        emb = (
            ((in0.view(np_.uint32) | PAYLOAD_MASK) ^ PAYLOAD_MASK)
            | in1.view(np_.uint32)
        ).view(np_.float32)
        return np_.maximum.accumulate(emb, axis=-1)

    def reg_embmax():
        name = "EMBMAX_SEG_ANT"
        for o in OPS:
            if o.name == name:
                return o
        seg = ds.Scan(op=AluOp.MAX, expr=embed_expr, init=None, _subdim_step=Zero)
        spec = Spec(body=seg, reference=_ref_embmax)
        orig_so, orig_nas = ds._scan_overrides, ds._node_as_stage

        def patched_so(scans, node_stage):
            seed, step = {}, {}
            for scan in scans:
                d = node_stage[scan]
                init = (
                    scan.init
                    if scan.init is not None
                    else ds._ACCUM_IDENTITY[scan.op]
                )
                seed[d] = orig_nas(init)
                if scan._subdim_step is not None:
                    step[d] = ds._Stage(AluOp.BYPASS, scan.expr)
            return seed, step

        def patched_nas(e):
            if isinstance(e, ds.Scan) and e._subdim_step is not None:
                return ds._Stage(e.op, ds.AluInp.CURR_ALU_OUT, e.expr)
            return orig_nas(e)

        uops_by_ver, shas = {}, {}
        ds._scan_overrides, ds._node_as_stage = patched_so, patched_nas
        try:
            for ver in ("v3", "v4"):
                uops_by_ver[ver] = lower(spec, ver=ver)
        finally:
            ds._scan_overrides, ds._node_as_stage = orig_so, orig_nas
        opcode = _CUSTOM_DVE_ROW_BASE + len(OPS)
        for ver in ("v3", "v4"):
            s = DveOpSpec(name=name, opcode=opcode, uops=uops_by_ver[ver], rd1_en=True)
            shas[ver] = s.sha(ver)
            _COMPILE_CACHE[(name, ver)] = s
        op = DveOp(name, spec, subdim=True, uops_sha=shas)
        _SUB_OPCODE_FOR_NAME[name] = opcode
        OPS.append(op)
        CUSTOM_DVE_SPECS[name] = spec
        return op

    def _uprime(src):
        return Bin(AluOp.BITWISE_OR, Bin(AluOp.BITWISE_AND, src, C0), C1)

    def _np_uprime(x):
        return (
            (x.view(np_.uint32) & PAYLOAD_MASK) | np_.uint32(0x4B000000)
        ).view(np_.float32)

    up_e, vp_e = _uprime(Src0), _uprime(Src1)
    wnum_spec = Spec(
        body=Bin(
            AluOp.MULTIPLY,
            ne(up_e, vp_e),
            Bin(AluOp.SUBTRACT, maxx(up_e, vp_e), C1),
        ),
        reference=lambda in0, in1, s0, s1, imm2: np_.where(
            _np_uprime(in0) != _np_uprime(in1),
            np_.maximum(_np_uprime(in0), _np_uprime(in1)) - np_.float32(F_2P23),
            np_.float32(0.0),
        ).astype(np_.float32),
    )
    spd_spec = Spec(
        body=Bin(
            AluOp.SUBTRACT, Bin(AluOp.ADD, up_e, vp_e), C2
        ),
        reference=lambda in0, in1, s0, s1, imm2: (
            _np_uprime(in0) + _np_uprime(in1) - np_.float32(F_2P24)
        ).astype(np_.float32),
    )

    embed = reg_embmax()
    wnum = reg("WNUM_ANT", wnum_spec, rd1=True)
    spd = reg("SPD_ANT", spd_spec, rd1=True)
    return embed, wnum, spd


def _emit_tile(nc, pools, pred_v, targ_v, pay_b, t, embed_op, mask_ap,
               me2, mt2, s2, half):
    """Streaming part for one [128, R*16] tile; row stats land in column
    block `half` of the stats tiles."""
    io_pool, work_pool, small_pool = pools
    cols = slice(half * R, (half + 1) * R)

    xp = io_pool.tile([P, F], f32, tag="xp")
    nc.sync.dma_start(out=xp[:, :], in_=pred_v[t])
    xt = io_pool.tile([P, F], f32, tag="xt")
    nc.scalar.dma_start(out=xt[:, :], in_=targ_v[t])

    # fused embed + segmented max over RAW predict on DVE; runs concurrently
    # with the exp pass on ACT (both only read xp)
    xp3 = xp[:, :].rearrange("p (r w) -> p r w", w=W)
    nc.vector._custom_dve(
        embed_op,
        out=me2[:, cols].unsqueeze(2).broadcast_to([P, R, W]),
        in0=xp3, in1=pay_b, s0=mask_ap,
    )

    # E = exp(predict) on ScalarE into a separate bf16 tile (xp stays raw).
    # bf16 halves Q7 bytes/elem for the tree below; sum rounding noise is
    # ~2^-8 per level, unbiased — far inside the 2e-2 budget.
    e = work_pool.tile([P, F], bf16, tag="e")
    nc.scalar.activation(e[:, :], xp[:, :], ACT.Exp)

    # row sums of E via contiguous-halves pairwise-add tree on GPSIMD
    e3 = e[:, :].rearrange("p (r w) -> p r w", w=W)
    l1 = work_pool.tile([P, R * 8], bf16, tag="l1")
    l1v = l1[:, :].rearrange("p (r h) -> p r h", h=8)
    nc.gpsimd.tensor_tensor(l1v, e3[:, :, 0:8], e3[:, :, 8:16], op=OP.add)
    l2 = work_pool.tile([P, R * 4], bf16, tag="l2")
    l2v = l2[:, :].rearrange("p (r h) -> p r h", h=4)
    nc.gpsimd.tensor_tensor(l2v, l1v[:, :, 0:4], l1v[:, :, 4:8], op=OP.add)
    l3 = work_pool.tile([P, R * 2], bf16, tag="l3")
    l3v = l3[:, :].rearrange("p (r h) -> p r h", h=2)
    nc.gpsimd.tensor_tensor(l3v, l2v[:, :, 0:2], l2v[:, :, 2:4], op=OP.add)
    # final level converts back to f32 for the formula
    nc.gpsimd.tensor_tensor(
        s2[:, cols].unsqueeze(2), l3v[:, :, 0:1], l3v[:, :, 1:2], op=OP.add
    )

    # target side: fused embed + segmented max on DVE
    xt3 = xt[:, :].rearrange("p (r w) -> p r w", w=W)
    nc.vector._custom_dve(
        embed_op,
        out=mt2[:, cols].unsqueeze(2).broadcast_to([P, R, W]),
        in0=xt3, in1=pay_b, s0=mask_ap,
    )


def _emit_formula(nc, small_pool, acc_sl, me2, mt2, s2, ops, mask_ap,
                  suffix, on_dve=False):
    """Per-row tail on a [128, width] stats block:
      loss = WNUM(me,mt) * exp(m) / (SPD(me,mt) * sumexp)."""
    _, wnum_op, spd_op = ops
    width = me2.shape[1]
    mul_eng = nc.vector if on_dve else nc.gpsimd

    wn = small_pool.tile([P, width], f32, tag="wn" + suffix)
    nc.vector._custom_dve(wnum_op, out=wn[:, :], in0=me2[:, :], in1=mt2[:, :],
                          s0=mask_ap, s1=F_2P23)
    sp = small_pool.tile([P, width], f32, tag="sp" + suffix)
    nc.vector._custom_dve(spd_op, out=sp[:, :], in0=me2[:, :], in1=mt2[:, :],
                          s0=mask_ap, s1=F_2P23, imm2=F_2P24)
    # em = exp(m): payload bits perturb m by <= 2^-14 relative — in budget
    em = small_pool.tile([P, width], f32, tag="em" + suffix)
    nc.scalar.activation(em[:, :], me2[:, :], ACT.Exp)

    den = small_pool.tile([P, width], f32, tag="dn" + suffix)
    mul_eng.tensor_tensor(den[:, :], sp[:, :], s2[:, :], op=OP.mult)
    rec = small_pool.tile([P, width], f32, tag="rc" + suffix)
    nc.vector.reciprocal_approx_fast(out=rec[:, :], in_=den[:, :])
    mul_eng.tensor_tensor(wn[:, :], wn[:, :], em[:, :], op=OP.mult)
    mul_eng.tensor_tensor(wn[:, :], wn[:, :], rec[:, :], op=OP.mult)
    mul_eng.tensor_tensor(acc_sl, acc_sl, wn[:, :], op=OP.add)


def _emit_pass(nc, pools, pred_v, targ_v, pay_b, acc, ops, mask_ap):
    small_pool = pools[2]
    embed_op = ops[0]
    # big block: tiles 0..BLK_A-1; its formula overlaps the tail tiles' DMA
    meA = small_pool.tile([P, BLK_A * R], f32, tag="meA")
    mtA = small_pool.tile([P, BLK_A * R], f32, tag="mtA")
    sA = small_pool.tile([P, BLK_A * R], f32, tag="sA")
    for half in range(BLK_A):
        _emit_tile(nc, pools, pred_v, targ_v, pay_b, half, embed_op, mask_ap,
                   meA, mtA, sA, half)
    _emit_formula(nc, small_pool, acc[:, : BLK_A * R], meA, mtA, sA, ops,
                  mask_ap, "A", on_dve=True)
    # tail block: tiles BLK_A..NT-1; formula on DVE (idle at the tail)
    meB = small_pool.tile([P, BLK_B * R], f32, tag="meB")
    mtB = small_pool.tile([P, BLK_B * R], f32, tag="mtB")
    sB = small_pool.tile([P, BLK_B * R], f32, tag="sB")
    for half in range(BLK_B):
        _emit_tile(nc, pools, pred_v, targ_v, pay_b, BLK_A + half, embed_op,
                   mask_ap, meB, mtB, sB, half)
    _emit_formula(nc, small_pool, acc[:, : BLK_B * R], meB, mtB, sB, ops,
                  mask_ap, "B", on_dve=True)


def _build_program(passes=1, dyn_iters=False):
    nc = bacc.Bacc("TRN2", target_bir_lowering=False, debug=False)
    pred = nc.dram_tensor("predict", [BS, W], f32, kind="ExternalInput")
    targ = nc.dram_tensor("target", [BS, W], f32, kind="ExternalInput")
    pay = nc.dram_tensor("payload", [P, W], u32, kind="ExternalInput")
    if dyn_iters:
        nit = nc.dram_tensor("niter", [1, 1], mybir.dt.int32, kind="ExternalInput")
    out = nc.dram_tensor("out", [P, BLK_A * R], f32, kind="ExternalOutput")

    pred_v = pred[:, :].rearrange("(t p r) w -> t p (r w)", t=NT, p=P, r=R)
    targ_v = targ[:, :].rearrange("(t p r) w -> t p (r w)", t=NT, p=P, r=R)

    with tile.TileContext(nc) as tc:
        with (
            tc.tile_pool(name="io", bufs=2) as io_pool,
            tc.tile_pool(name="work", bufs=2) as work_pool,
            tc.tile_pool(name="small", bufs=1) as small_pool,
            tc.tile_pool(name="const", bufs=1) as const_pool,
        ):
            pay_t = const_pool.tile([P, W], u32, tag="pay")
            nc.gpsimd.dma_start(out=pay_t[:, :], in_=pay[:, :])
            pay_b = pay_t[:, :].unsqueeze(1).broadcast_to([P, R, W]).bitcast(f32)

            mask_t = const_pool.tile([P, 1], u32, tag="mask")
            nc.vector.memset(mask_t[:, :], PAYLOAD_MASK)
            mask_ap = mask_t[:, :1].bitcast(f32)

            acc = const_pool.tile([P, BLK_A * R], f32, tag="acc")
            nc.vector.memset(acc[:, :], 0.0)

            ops = _register_custom_ops()
            pools = (io_pool, work_pool, small_pool)
            if dyn_iters:
                from concourse.bass import RegisterHandles, make_scalar_value

                nit_t = const_pool.tile([1, 1], mybir.dt.int32, tag="nit")
                nc.gpsimd.dma_start(out=nit_t[:, :], in_=nit[:, :])
                regs = []
                for ename, eng in nc.engines.items():
                    r = eng.alloc_register(f"nit_{ename}")
                    eng.reg_load(r, nit_t[:1, :1])
                    regs.append(r)
                n = make_scalar_value(RegisterHandles(regs), min_val=1, max_val=1024)
                with tc.For_i(0, n, 1):
                    _emit_pass(nc, pools, pred_v, targ_v, pay_b, acc, ops, mask_ap)
            else:
                for _ in range(passes):
                    _emit_pass(nc, pools, pred_v, targ_v, pay_b, acc, ops, mask_ap)

            nc.sync.dma_start(out=out[:, :], in_=acc[:, :])
    nc.compile()
    return nc


_CACHE = {}


def _run(predict, target, trace=False):
    if "nc" not in _CACHE:
        _CACHE["nc"] = _build_program()
    nc = _CACHE["nc"]

    predict = np.ascontiguousarray(np.asarray(predict, dtype=np.float32))
    target = np.ascontiguousarray(np.asarray(target, dtype=np.float32))
    payload = np.broadcast_to(
        (np.asarray(LABELS_NUM_COUNT, dtype=np.uint32) // 1000)[None, :], (P, W)
    ).copy()

    in_maps = []
    for i in range(NCORES):
        in_maps.append(
            {
                "predict": predict[i * BS : (i + 1) * BS],
                "target": target[i * BS : (i + 1) * BS],
                "payload": payload,
            }
        )
    res = run_bass_kernel_spmd(nc, in_maps, core_ids=list(range(NCORES)), trace=trace)
    total = np.float64(0.0)
    for r in res.results:
        total += np.float64(r["out"].astype(np.float64).sum())
    value = np.float32(total / B)
    return np.asarray(value, dtype=np.float32), res


def kernel(predict, target, penalty_matrix=None):
    value, _ = _run(predict, target, trace=False)
    return value

